# revision 30
# baseline (speedup 1.0000x reference)
"""ArcFace fully-connected loss head on 8 Trainium2 NeuronCores.

Computes  out = s * (onehot(label) * phi + (1-onehot) * cos)  where
cos = l2norm(x) @ l2norm(W).T, phi = cos(arccos(cos)+m) with the ArcFace
threshold branch.

Distribution: classification-parallel (Partial-FC style). The class dim
C=100000 is split into 8 contiguous shards of 12500 (padded to 12544);
every core gets the normalized input replicated (per the sharding hint)
pre-transposed to [D, B] bf16, plus its weight shard pre-normalized,
cast to bf16, and pre-transposed on the host into the
[d-partition, kd, class] layout the matmul consumes directly.

Device pipeline per core (PE floor 84us at 1 bf16 row/cycle vs ~79us
in+out DMA floor at ~325GB/s; measured PE ~97.5% busy in its window):
  - DMA in: one interleaved DMA per class chunk (row = j*128 + p of
    2KB), the access pattern that splits across all 16 SDMA engines
    (~325GB/s) instead of 5 (~119GB/s) for a linear range; 12.8MB/core.
  - The shard is cut as [256, 256, 512 x 23, 256] classes: a small first
    chunk (fast PE start behind a ~380KB dependency) and a small last
    chunk (short evac+store drain). The replicated input is split into
    four per-batch-block tiles, all issued before the second chunk so
    the PE never waits on them.
  - PE: pure bf16 matmuls accumulating over D into PSUM, all 8 banks;
    no transposes, no casts - the host did both. (No PE "warm-up" ops:
    touching the PE during the NEFF init window locks the DVFS governor
    at 2.0GHz instead of 2.4GHz for the whole run.)
  - ACT/DVE alternate evacuating PSUM banks (x30 scale + f32->bf16) into
    shared tiles spanning a class-adjacent chunk pair; the engine that
    finishes a pair issues its store (ACT for even batch blocks, SP for
    odd ones) so neither in-order sequencer serializes the drain;
    12.8MB/core out.
  - ArcFace margin only changes the single label column per row (512 of
    51.2M elements): host applies it to the returned s*cos values.
"""

import math
import sys

sys.path.insert(0, "/opt/trn_rl_repo")

import numpy as np

B, D, C = 512, 512, 100000
N_CORES = 8
CL = C // N_CORES      # 12500 classes per core
CLP = 12544            # padded
KD = D // 128          # 4 contraction blocks
NB = B // 128          # 4 batch blocks
# chunk class sizes, processed in order; all pairs (0,1), (2,3), ... are
# class-adjacent so each pair shares one output tile and store
CHUNKS = [256, 256] + [512] * 23 + [256]
# DRAM rows of 1024 bf16 (2KB) per core; the first two 256-row spans each
# hold a small weight chunk plus two x blocks
NROWS = 512 + sum(n // 2 for n in CHUNKS[2:])
S_SCALE = 30.0
MARGIN = 0.5
COS_M = math.cos(MARGIN)
SIN_M = math.sin(MARGIN)
TH = math.cos(math.pi - MARGIN)
MM = math.sin(math.pi - MARGIN) * MARGIN
EPS = 1e-12

_CACHE = {}


def _build():
    if "nc" in _CACHE:
        return _CACHE["nc"]
    from contextlib import ExitStack

    import concourse.mybir as mybir
    import concourse.tile as tile
    from concourse import bacc

    f32 = mybir.dt.float32
    bf16 = mybir.dt.bfloat16
    AF = mybir.ActivationFunctionType

    nc = bacc.Bacc("TRN2", target_bir_lowering=False)
    wt_d = nc.dram_tensor("wt", [NROWS, 1024], bf16, kind="ExternalInput")
    o_d = nc.dram_tensor("out", [B, CLP], bf16, kind="ExternalOutput")

    with tile.TileContext(nc) as tc, ExitStack() as ctx:
        wpool = ctx.enter_context(tc.tile_pool(name="wpool", bufs=16))
        outpool = ctx.enter_context(tc.tile_pool(name="outpool", bufs=12))
        mmpsum = ctx.enter_context(tc.tile_pool(name="mmpsum", bufs=8, space="PSUM"))

        # rows: [c0a | x01] [c0b | x23] fulls... tail; the first two DMAs
        # carry both a small weight chunk and two x blocks, so the first
        # matmul's whole dependency is one DMA
        row0 = [0, 256] + [512 + sum(n // 2 for n in CHUNKS[2:i]) for i in range(2, 26)]
        c0s = [sum(CHUNKS[:i]) for i in range(len(CHUNKS))]

        def load_span(i, nrows, tag, bufs):
            wt = wpool.tile([128, nrows // 128, 1024], bf16, tag=tag, bufs=bufs)
            nc.sync.dma_start(
                out=wt,
                in_=wt_d[row0[i] : row0[i] + nrows, :].rearrange(
                    "(j p) w -> p j w", p=128
                ),
                max_dma_last_dim=1024,
            )
            return wt

        xnT = [None] * NB
        tiles = {}
        tiles[0] = load_span(0, 256, "wx", 2)
        xnT[0] = tiles[0][:, 1, :512]
        xnT[1] = tiles[0][:, 1, 512:]
        tiles[1] = load_span(1, 256, "wx", 2)
        xnT[2] = tiles[1][:, 1, :512]
        xnT[3] = tiles[1][:, 1, 512:]
        for i in range(2, 26):
            n = CHUNKS[i]
            tiles[i] = load_span(
                i, n // 2, "wt2" if n == 512 else "wt1", 16 if n == 512 else 3
            )

        def mv(wt, i, kd):
            if CHUNKS[i] == 512:
                return wt[:, kd // 2, (kd % 2) * 512 : (kd % 2) * 512 + 512]
            return wt[:, 0, kd * 256 : (kd + 1) * 256]

        ot_live = {}
        for i, n in enumerate(CHUNKS):
            wt = tiles[i]
            pn = n + CHUNKS[i + 1] if i % 2 == 0 else CHUNKS[i - 1] + n
            off = 0 if i % 2 == 0 else CHUNKS[i - 1]
            for bi in range(NB):
                po = mmpsum.tile([128, 512], f32, tag="po")
                for kd in range(KD):
                    nc.tensor.matmul(
                        po[:, :n],
                        xnT[bi][:, kd * 128 : (kd + 1) * 128],
                        mv(wt, i, kd),
                        start=(kd == 0),
                        stop=(kd == KD - 1),
                    )
                if i >= 24:
                    # the final two chunks store per-chunk so the last store
                    # is small and issues right after its evac
                    ot = outpool.tile([128, n], bf16, tag=f"otf{n}", bufs=4)
                    off = 0
                else:
                    if i % 2 == 0:
                        ot = outpool.tile(
                            [128, pn], bf16, tag=f"ot{pn}", bufs=12 if pn == 1024 else 4
                        )
                        ot_live[bi] = ot
                    else:
                        ot = ot_live.pop(bi)
                if bi % 2 == 0:
                    nc.scalar.activation(
                        out=ot[:, off : off + n], in_=po[:, :n], func=AF.Copy,
                        scale=S_SCALE,
                    )
                else:
                    nc.vector.tensor_scalar_mul(ot[:, off : off + n], po[:, :n], S_SCALE)
                if i >= 24 or i % 2 == 1:
                    eng = nc.scalar if bi % 2 == 0 else nc.sync
                    lo = c0s[i] if i >= 24 else c0s[i - 1]
                    eng.dma_start(
                        out=o_d[bi * 128 : (bi + 1) * 128, lo : c0s[i] + n], in_=ot
                    )

    nc.compile()
    _CACHE["nc"] = nc
    return nc


def _in_maps(x, w):
    import ml_dtypes

    bf = ml_dtypes.bfloat16
    # host-side prep mirrors the sharding hint: replicate the normalized
    # input; give each shard its (normalized) weight slice
    xn = x / np.maximum(
        np.sqrt(np.einsum("bd,bd->b", x, x, dtype=np.float64)), EPS
    )[:, None].astype(np.float32)
    # xnt[bi, p, kd*128 + b'] = xn[bi*128 + b', kd*128 + p]
    xnt = xn.astype(bf).reshape(NB, 128, KD, 128).transpose(0, 3, 2, 1).reshape(
        NB, 128, KD * 128
    )
    x01 = np.concatenate([xnt[0], xnt[1]], axis=1)  # [128, 1024] row block
    x23 = np.concatenate([xnt[2], xnt[3]], axis=1)

    wnorm = np.maximum(
        np.sqrt(np.einsum("cd,cd->c", w, w, dtype=np.float64)), EPS
    ).astype(np.float32)
    in_maps = []
    for k in range(N_CORES):
        wk = w[k * CL : (k + 1) * CL] / wnorm[k * CL : (k + 1) * CL, None]
        wn = np.zeros((CLP, D), dtype=bf)
        wn[:CL] = wk.astype(bf)
        rows = []
        c0 = 0
        for i, n in enumerate(CHUNKS):
            blk = wn[c0 : c0 + n]
            if n == 512:
                # full: row (j*128 + p)[k2*512 + c] = blk[c, (2j + k2)*128 + p]
                rows.append(
                    blk.reshape(512, KD, 128)
                    .transpose(1, 2, 0)  # [kd, p, c]
                    .reshape(2, 2, 128, 512)
                    .transpose(0, 2, 1, 3)  # [j, p, k2, c]
                    .reshape(256, 1024)
                )
            else:
                # small: row p[kd*256 + c] = blk[c, kd*128 + p]
                rows.append(
                    blk.reshape(256, KD, 128).transpose(1, 2, 0)  # [kd, p, c]
                    .transpose(1, 0, 2)  # [p, kd, c]
                    .reshape(128, 1024)
                )
            if i == 0:
                rows.append(x01)
            elif i == 1:
                rows.append(x23)
            c0 += n
        wt = np.ascontiguousarray(np.concatenate(rows, axis=0))
        in_maps.append({"wt": wt})
    return in_maps


def kernel(input, weight, label):
    from concourse.bass_utils import run_bass_kernel_spmd

    nc = _build()
    x = np.ascontiguousarray(np.asarray(input, dtype=np.float32))
    w = np.ascontiguousarray(np.asarray(weight, dtype=np.float32))
    res = run_bass_kernel_spmd(nc, _in_maps(x, w), core_ids=list(range(N_CORES)))
    out = np.concatenate(
        [res.results[k]["out"][:, :CL] for k in range(N_CORES)], axis=1
    ).astype(np.float32)

    # ArcFace margin on the label column of each row (device emitted s*cos)
    rows = np.arange(B)
    cols = np.asarray(label).astype(np.int64)
    cos = out[rows, cols].astype(np.float64) / S_SCALE
    sine = np.sqrt(np.maximum(0.0, 1.0 - cos * cos))
    phi = cos * COS_M - sine * SIN_M
    phi = np.where(cos > TH, phi, cos - MM)
    out[rows, cols] = (phi * S_SCALE).astype(np.float32)
    return out
